# revision 46
# baseline (speedup 1.0000x reference)
"""Trainium2 Bass kernel for a 2-layer LSTM encoder + autoregressive decoder.

Problem: batch 8192, 48 encode steps, 12 decode steps with BG-channel
feedback, hidden 128, input dim 8, fc head to 1 output.

Strategy: pure data parallelism over 8 NeuronCores (1024 batch rows each).
Per core the recurrence runs sequentially; within a step, hidden units sit
on SBUF partitions and batch (1024) on the free dim:

  gates[512, B] = W_ih^T.T @ x[9, B] + W_hh^T.T @ h[128, B]   (PSUM accum)

Each gate (i, f, g, o) is one 128-partition tile. The ISA caps the moving
free dim at 512, so every matmul handles one 512-wide half ("stream") of
the batch; both halves share each loaded weight back-to-back (long PE runs
keep the HAM clock-gate warm) and all activation/elementwise ops process
both halves in a single instruction on [128, 2, 512] tensors.

PSUM layout per layer-step: one 4-bank tile [128, (i,f), 2, 512] and one
[128, (o,g), 2, 512]. Layer 0's bias is folded into a 9th constant-one
input channel; layer 1's bias uses the activation instruction's
per-partition bias operand (per-gate ops). The decode BG feedback is a K=1
accumulating matmul from a [1, 2, 512] SBUF tile overwritten with the fc
output each step.

Matmuls run in bf16 (fp32 PSUM accumulation); gates and cell state are
fp16 (DVE 2x mode); h is bf16 (matmul rhs). h is double-buffered on step
parity so next-step matmuls can overlap this step's tail.
"""

import sys

sys.path.insert(0, "/opt/trn_rl_repo")

import numpy as np
import ml_dtypes

import concourse.bacc as bacc
import concourse.tile as tile
from concourse import mybir
from concourse import bass_utils
from concourse.bass import ts

BF16 = ml_dtypes.bfloat16

B_TOTAL = 8192
T = 60
T_ENC = 48
T_DEC = 12
DIN = 8
H = 128
NG = 4 * H
N_CORES = 8
BSH = B_TOTAL // N_CORES  # 1024 batch rows per core
NS = 2  # batch halves (matmul moving-dim limit is 512)
SB = BSH // NS  # 512
XT_STEPS = 8  # timesteps per streamed x tile

# pytorch gate order in the weight columns: i, f, g, o
GI, GF, GG, GO = 0, 1, 2, 3

_CACHE: dict = {}


def _build(bfc: float):
    f32 = mybir.dt.float32
    f16 = mybir.dt.float16
    bf16 = mybir.dt.bfloat16

    nc = bacc.Bacc("TRN2", debug=False, num_devices=N_CORES)

    x_d = nc.dram_tensor("x", [DIN + 1, T, BSH], bf16, kind="ExternalInput")
    w9t0_d = nc.dram_tensor("w9t0", [DIN + 1, NG], bf16, kind="ExternalInput")
    w9dec_d = nc.dram_tensor("w9dec", [DIN + 1, NG], bf16, kind="ExternalInput")
    whht0_d = nc.dram_tensor("whht0", [H, NG], bf16, kind="ExternalInput")
    wiht1_d = nc.dram_tensor("wiht1", [H, NG], bf16, kind="ExternalInput")
    whht1_d = nc.dram_tensor("whht1", [H, NG], bf16, kind="ExternalInput")
    wbg0t_d = nc.dram_tensor("wbg0t", [1, NG], bf16, kind="ExternalInput")
    wfct_d = nc.dram_tensor("wfct", [H, 1], bf16, kind="ExternalInput")
    b1_d = nc.dram_tensor("b1", [H, 4], f32, kind="ExternalInput")
    bg0_d = nc.dram_tensor("bg0", [1, BSH], bf16, kind="ExternalInput")
    out_d = nc.dram_tensor("out", [T_DEC, BSH], f32, kind="ExternalOutput")

    SIG = mybir.ActivationFunctionType.Sigmoid
    TANH = mybir.ActivationFunctionType.Tanh

    with tile.TileContext(nc) as tc:
        with (
            tc.tile_pool(name="wpool", bufs=1) as wpool,
            tc.tile_pool(name="xpool", bufs=3) as xpool,
            tc.tile_pool(name="state", bufs=1) as state,
            tc.tile_pool(name="gates", bufs=3) as gates,
            tc.tile_pool(name="psum", bufs=2, space="PSUM") as psum,
        ):
            w9t0 = wpool.tile([DIN + 1, NG], bf16)
            w9dec = wpool.tile([DIN + 1, NG], bf16)
            whht0 = wpool.tile([H, NG], bf16)
            wiht1 = wpool.tile([H, NG], bf16)
            whht1 = wpool.tile([H, NG], bf16)
            wbg0t = wpool.tile([1, NG], bf16)
            wfct = wpool.tile([H, 1], bf16)
            b1 = wpool.tile([H, 4], f32)
            nc.sync.dma_start(w9t0[:], w9t0_d.ap())
            nc.sync.dma_start(w9dec[:], w9dec_d.ap())
            nc.sync.dma_start(whht0[:], whht0_d.ap())
            nc.sync.dma_start(wiht1[:], wiht1_d.ap())
            nc.sync.dma_start(whht1[:], whht1_d.ap())
            nc.sync.dma_start(wbg0t[:], wbg0t_d.ap())
            nc.sync.dma_start(wfct[:], wfct_d.ap())
            nc.sync.dma_start(b1[:], b1_d.ap())

            # recurrent state; h double-buffered on step parity
            h = [[None, None], [None, None]]  # h[layer][parity]
            c = [None, None]  # c[layer]
            for l in range(2):
                for p in range(2):
                    h[l][p] = state.tile([H, NS, SB], bf16, name=f"h_{l}_{p}")
                    nc.vector.memset(h[l][p][:], 0.0)
                c[l] = state.tile([H, NS, SB], f16, name=f"c_{l}")
                nc.vector.memset(c[l][:], 0.0)
            bg = state.tile([1, NS, SB], bf16, name="bg")
            nc.sync.dma_start(bg[:], bg0_d.ap())

            def layer_mm(t, layer, xt, tr):
                """Emit this layer's matmuls, gate-major.

                Each gate gets its own 2-bank PSUM tile [128, stream, 512];
                four tiles are in flight per layer (2 tags x 2 bufs = all 8
                banks), so PSUM frees one sigma/tanh op at a time and the
                tensor engine never faces a whole-layer drain bubble.
                Within a gate, the part whose rhs only needs last-step state
                goes first so the scheduler can prefetch it.
                """
                p = t % 2
                dec = t >= T_ENC
                if layer == 0:
                    # t == T_ENC uses the real BG value (bg0) with the normal
                    # bias row; later decode steps feed back pred = fc + b_fc,
                    # where b_fc * W_bg is folded into w9dec's bias row so the
                    # feedback copy is a raw PSUM -> SBUF copy.
                    w_x = w9dec if t > T_ENC else w9t0
                else:
                    w_x = wiht1
                w_h = whht0 if layer == 0 else whht1

                def x_sl(st):
                    if layer == 0:
                        return xt[:, tr, ts(st, SB)]
                    return h[0][p][:, st, :]

                def h_sl(st):
                    return h[layer][1 - p][:, st, :]

                if layer == 0:
                    parts = [(w_x, x_sl), (w_h, h_sl)]
                    if dec:
                        # BG feedback rides a late K=1 matmul so the rest of
                        # the layer stays prefetchable
                        parts.append((wbg0t, lambda st: bg[:, st, :]))
                else:
                    # h-part first: it only needs last-step h1 (prefetchable);
                    # the x-part (= this step's h0) is the late dependency
                    parts = [(w_h, h_sl), (w_x, x_sl)]

                # g first (it unblocks the DVE chain), then i, f, o
                gps = {}
                for g, tag in [(GG, "psA"), (GI, "psB"), (GF, "psA"), (GO, "psB")]:
                    gps[g] = psum.tile(
                        [H, NS, SB], f32, tag=tag, name=f"ps_{t}_{layer}_{g}"
                    )
                if dec:
                    # stream-major: stream A's whole chain races ahead of B's
                    # (decode is latency-bound, engines are half idle)
                    for st in range(NS):
                        for pi, (w, rhs_fn) in enumerate(parts):
                            for g in (GG, GI, GF, GO):
                                nc.tensor.matmul(
                                    gps[g][:, st, :], w[:, ts(g, H)], rhs_fn(st),
                                    start=pi == 0, stop=pi == len(parts) - 1,
                                )
                else:
                    for pi, (w, rhs_fn) in enumerate(parts):
                        for g in (GG, GI, GF, GO):
                            for st in range(NS):
                                nc.tensor.matmul(
                                    gps[g][:, st, :], w[:, ts(g, H)], rhs_fn(st),
                                    start=pi == 0, stop=pi == len(parts) - 1,
                                )
                return gps

            def layer_act_dve(t, layer, gps):
                p = t % 2
                dec = t >= T_ENC
                h_new = h[layer][p]
                c_own = c[layer]

                ifo_sb = gates.tile([H, 3, NS, SB], f16, tag="ifo")
                g_sb = gates.tile([H, NS, SB], f16, tag="g")
                t1 = gates.tile([H, NS, SB], f16, tag="t1")
                u = gates.tile([H, NS, SB], f16, tag="u")
                th = gates.tile([H, NS, SB], f16, tag="th")
                bias = {}
                if layer == 1:
                    bias = {
                        g: {"bias": b1[:, g : g + 1]} for g in (GI, GF, GG, GO)
                    }

                # decode is latency-bound: emit per-stream (half-width) ops so
                # stream A's chain completes early; encode is ACT-throughput-
                # bound: emit full-width ops (less per-op overhead)
                strs = [(st, st + 1) for st in range(NS)] if dec else [(0, NS)]
                for lo, hi in strs:
                    sl = slice(lo, hi)
                    nc.scalar.activation(
                        g_sb[:, sl], gps[GG][:, sl], TANH, **bias.get(GG, {})
                    )
                    nc.scalar.activation(
                        ifo_sb[:, 0, sl], gps[GI][:, sl], SIG, **bias.get(GI, {})
                    )
                    nc.scalar.activation(
                        ifo_sb[:, 1, sl], gps[GF][:, sl], SIG, **bias.get(GF, {})
                    )
                    nc.vector.tensor_mul(t1[:, sl], ifo_sb[:, 0, sl], g_sb[:, sl])
                    nc.vector.tensor_mul(u[:, sl], ifo_sb[:, 1, sl], c_own[:, sl])
                    nc.vector.tensor_add(c_own[:, sl], u[:, sl], t1[:, sl])
                    nc.scalar.activation(th[:, sl], c_own[:, sl], TANH)
                    nc.scalar.activation(
                        ifo_sb[:, 2, sl], gps[GO][:, sl], SIG, **bias.get(GO, {})
                    )
                    nc.vector.tensor_mul(
                        h_new[:, sl], ifo_sb[:, 2, sl], th[:, sl]
                    )

            def fc_block(t, xt, tr):
                td = t - T_ENC
                fc = psum.tile([1, NS, SB], f32, tag="psA", name=f"fc_{t}")
                for st in range(NS):
                    nc.tensor.matmul(
                        fc[:, st, :], wfct[:], h[1][t % 2][:, st, :],
                        start=True, stop=True,
                    )
                    if td + 1 < T_DEC:
                        # BG feedback (critical path): raw per-stream copy,
                        # b_fc is folded into w9dec's bias row
                        nc.vector.tensor_copy(bg[:, st, :], fc[:, st, :])
                # output staging (off the critical path): add b_fc, DMA out
                pred = gates.tile([1, NS, SB], f32, tag="pred")
                nc.vector.tensor_scalar_add(pred[:], fc[:], bfc)
                nc.sync.dma_start(out_d.ap()[td : td + 1, :], pred[:])

            # x tiles: 8-step tiles for encode, one 12-step tile for the
            # whole decode window (so the BG feedback write target exists)
            xt = None
            x_tile_starts = list(range(0, T_ENC, XT_STEPS)) + [T_ENC]
            for t in range(T):
                if t in x_tile_starts:
                    t0 = t
                    nt = T_DEC if t == T_ENC else XT_STEPS
                    xt = xpool.tile([DIN + 1, T_DEC, BSH], bf16)
                    nc.sync.dma_start(
                        xt[:, :nt, :], x_d.ap()[:, t : t + nt, :]
                    )
                tr = t - t0
                for layer in range(2):
                    gps = layer_mm(t, layer, xt, tr)
                    layer_act_dve(t, layer, gps)
                if t >= T_ENC:
                    fc_block(t, xt, tr)

    nc.compile()
    return nc


def _get_nc(bfc: float):
    if _CACHE.get("bfc") != bfc:
        _CACHE["nc"] = _build(bfc)
        _CACHE["bfc"] = bfc
    return _CACHE["nc"]


def kernel(
    inputs,
    W_ih_0, W_hh_0, b_ih_0, b_hh_0,
    W_ih_1, W_hh_1, b_ih_1, b_hh_1,
    W_fc, b_fc,
):
    inputs = np.asarray(inputs, np.float32)
    bfc = float(np.asarray(b_fc).reshape(-1)[0])
    nc = _get_nc(bfc)

    b0 = (b_ih_0 + b_hh_0).astype(np.float32)
    bfc32 = np.float32(bfc)
    w9t0 = np.concatenate(
        [W_ih_0.T.astype(np.float32), b0[None, :]], axis=0
    ).astype(BF16)  # [9, 512]; row 8 is the bias
    # decode variant: bias row also carries b_fc * W_bg (the feedback copy
    # delivers the raw fc output, without b_fc)
    w9dec = np.concatenate(
        [W_ih_0.T.astype(np.float32),
         (b0 + bfc32 * W_ih_0[:, 0].astype(np.float32))[None, :]], axis=0
    ).astype(BF16)
    whht0 = W_hh_0.T.astype(BF16)
    wiht1 = W_ih_1.T.astype(BF16)
    whht1 = W_hh_1.T.astype(BF16)
    wbg0t = W_ih_0.T[0:1, :].astype(BF16)  # BG column of W_ih_0
    wfct = W_fc.T.astype(BF16)  # [128, 1]
    b1 = (b_ih_1 + b_hh_1).reshape(4, H).T.astype(np.float32)  # [128, 4]

    in_maps = []
    for i in range(N_CORES):
        sh = inputs[i * BSH : (i + 1) * BSH]  # [1024, 60, 8]
        x = np.ascontiguousarray(sh.transpose(2, 1, 0))  # [8, 60, 1024]
        x9 = np.concatenate(
            [x, np.ones((1, T, BSH), np.float32)], axis=0
        )  # [9, 60, 1024]
        x9[0, T_ENC:, :] = 0.0  # BG channel rides the feedback matmul in decode
        bg0 = sh[:, T_ENC, 0].reshape(1, BSH)
        in_maps.append(
            {
                "x": x9.astype(BF16),
                "w9t0": w9t0,
                "w9dec": w9dec,
                "whht0": whht0,
                "wiht1": wiht1,
                "whht1": whht1,
                "wbg0t": wbg0t,
                "wfct": wfct,
                "b1": b1,
                "bg0": bg0.astype(BF16),
            }
        )

    res = bass_utils.run_bass_kernel_spmd(
        nc, in_maps, core_ids=list(range(N_CORES))
    )
    outs = []
    for i in range(N_CORES):
        o = res.results[i]["out"]  # [12, 1024] fp32, b_fc already added
        outs.append(o.T[:, :, None])  # [1024, 12, 1]
    return np.concatenate(outs, axis=0).astype(np.float32)


if __name__ == "__main__":
    _get_nc(0.0)
    print("build + compile OK")


# revision 54
# speedup vs baseline: 1.0614x; 1.0614x over previous
"""Trainium2 Bass kernel for a 2-layer LSTM encoder + autoregressive decoder.

Problem: batch 8192, 48 encode steps, 12 decode steps with BG-channel
feedback, hidden 128, input dim 8, fc head to 1 output.

Strategy: pure data parallelism over 8 NeuronCores (1024 batch rows each).
Per core the recurrence runs sequentially; within a step, hidden units sit
on SBUF partitions and batch (1024) on the free dim:

  gates[512, B] = W_ih^T.T @ x[9, B] + W_hh^T.T @ h[128, B]   (PSUM accum)

Each gate (i, f, g, o) is one 128-partition tile. The ISA caps the moving
free dim at 512, so every matmul handles one 512-wide half ("stream") of
the batch; both halves share each loaded weight back-to-back (long PE runs
keep the HAM clock-gate warm) and all activation/elementwise ops process
both halves in a single instruction on [128, 2, 512] tensors.

PSUM layout per layer-step: one 4-bank tile [128, (i,f), 2, 512] and one
[128, (o,g), 2, 512]. Layer 0's bias is folded into a 9th constant-one
input channel; layer 1's bias uses the activation instruction's
per-partition bias operand (per-gate ops). The decode BG feedback is a K=1
accumulating matmul from a [1, 2, 512] SBUF tile overwritten with the fc
output each step.

Matmuls run in bf16 (fp32 PSUM accumulation); gates and cell state are
fp16 (DVE 2x mode); h is bf16 (matmul rhs). h is double-buffered on step
parity so next-step matmuls can overlap this step's tail.
"""

import sys

sys.path.insert(0, "/opt/trn_rl_repo")

import numpy as np
import ml_dtypes

import concourse.bacc as bacc
import concourse.tile as tile
from concourse import mybir
from concourse import bass_utils
from concourse.bass import ts

BF16 = ml_dtypes.bfloat16

B_TOTAL = 8192
T = 60
T_ENC = 48
T_DEC = 12
DIN = 8
H = 128
NG = 4 * H
N_CORES = 8
BSH = B_TOTAL // N_CORES  # 1024 batch rows per core
NS = 2  # batch halves (matmul moving-dim limit is 512)
SB = BSH // NS  # 512
XT_STEPS = 8  # timesteps per streamed x tile

# pytorch gate order in the weight columns: i, f, g, o
GI, GF, GG, GO = 0, 1, 2, 3

_CACHE: dict = {}


def _build(bfc: float):
    f32 = mybir.dt.float32
    f16 = mybir.dt.float16
    bf16 = mybir.dt.bfloat16

    nc = bacc.Bacc("TRN2", debug=False, num_devices=N_CORES)

    x_d = nc.dram_tensor("x", [DIN + 1, T, BSH], bf16, kind="ExternalInput")
    w9t0_d = nc.dram_tensor("w9t0", [DIN + 1, NG], bf16, kind="ExternalInput")
    w9dec_d = nc.dram_tensor("w9dec", [DIN + 1, NG], bf16, kind="ExternalInput")
    whht0_d = nc.dram_tensor("whht0", [H, NG], bf16, kind="ExternalInput")
    wiht1_d = nc.dram_tensor("wiht1", [H, NG], bf16, kind="ExternalInput")
    whht1_d = nc.dram_tensor("whht1", [H, NG], bf16, kind="ExternalInput")
    wbg0t_d = nc.dram_tensor("wbg0t", [1, NG], bf16, kind="ExternalInput")
    wbgfc_d = nc.dram_tensor("wbgfc", [H, NG], bf16, kind="ExternalInput")
    wfct_d = nc.dram_tensor("wfct", [H, 1], bf16, kind="ExternalInput")
    b1_d = nc.dram_tensor("b1", [H, 4], f32, kind="ExternalInput")
    bg0_d = nc.dram_tensor("bg0", [1, BSH], bf16, kind="ExternalInput")
    out_d = nc.dram_tensor("out", [T_DEC, BSH], f32, kind="ExternalOutput")

    SIG = mybir.ActivationFunctionType.Sigmoid
    TANH = mybir.ActivationFunctionType.Tanh

    with tile.TileContext(nc) as tc:
        with (
            tc.tile_pool(name="wpool", bufs=1) as wpool,
            tc.tile_pool(name="xpool", bufs=3) as xpool,
            tc.tile_pool(name="state", bufs=1) as state,
            tc.tile_pool(name="gates", bufs=3) as gates,
            tc.tile_pool(name="psum", bufs=2, space="PSUM") as psum,
        ):
            w9t0 = wpool.tile([DIN + 1, NG], bf16)
            w9dec = wpool.tile([DIN + 1, NG], bf16)
            whht0 = wpool.tile([H, NG], bf16)
            wiht1 = wpool.tile([H, NG], bf16)
            whht1 = wpool.tile([H, NG], bf16)
            wbg0t = wpool.tile([1, NG], bf16)
            wbgfc = wpool.tile([H, NG], bf16)
            wfct = wpool.tile([H, 1], bf16)
            b1 = wpool.tile([H, 4], f32)
            nc.sync.dma_start(w9t0[:], w9t0_d.ap())
            nc.sync.dma_start(w9dec[:], w9dec_d.ap())
            nc.sync.dma_start(whht0[:], whht0_d.ap())
            nc.sync.dma_start(wiht1[:], wiht1_d.ap())
            nc.sync.dma_start(whht1[:], whht1_d.ap())
            nc.sync.dma_start(wbg0t[:], wbg0t_d.ap())
            nc.sync.dma_start(wbgfc[:], wbgfc_d.ap())
            nc.sync.dma_start(wfct[:], wfct_d.ap())
            nc.sync.dma_start(b1[:], b1_d.ap())

            # recurrent state; h double-buffered on step parity
            h = [[None, None], [None, None]]  # h[layer][parity]
            c = [None, None]  # c[layer]
            for l in range(2):
                for p in range(2):
                    h[l][p] = state.tile([H, NS, SB], bf16, name=f"h_{l}_{p}")
                    nc.vector.memset(h[l][p][:], 0.0)
                c[l] = state.tile([H, NS, SB], f16, name=f"c_{l}")
                nc.vector.memset(c[l][:], 0.0)
            bg = state.tile([1, NS, SB], bf16, name="bg")
            nc.sync.dma_start(bg[:], bg0_d.ap())

            def layer_mm(t, layer, xt, tr):
                """Emit this layer's matmuls, gate-major.

                Each gate gets its own 2-bank PSUM tile [128, stream, 512];
                four tiles are in flight per layer (2 tags x 2 bufs = all 8
                banks), so PSUM frees one sigma/tanh op at a time and the
                tensor engine never faces a whole-layer drain bubble.
                Within a gate, the part whose rhs only needs last-step state
                goes first so the scheduler can prefetch it.
                """
                p = t % 2
                dec = t >= T_ENC
                if layer == 0:
                    # t == T_ENC uses the real BG value (bg0) with the normal
                    # bias row; later decode steps feed back pred = fc + b_fc,
                    # where b_fc * W_bg is folded into w9dec's bias row so the
                    # feedback copy is a raw PSUM -> SBUF copy.
                    w_x = w9dec if t > T_ENC else w9t0
                else:
                    w_x = wiht1
                w_h = whht0 if layer == 0 else whht1

                def x_sl(st):
                    if layer == 0:
                        return xt[:, tr, ts(st, SB)]
                    return h[0][p][:, st, :]

                def h_sl(st):
                    return h[layer][1 - p][:, st, :]

                if layer == 0:
                    parts = [(w_x, x_sl), (w_h, h_sl)]
                    if t == T_ENC:
                        # first decode step: BG is the real input value (bg0)
                        parts.append((wbg0t, lambda st: bg[:, st, :]))
                    elif dec:
                        # later decode steps: pred_{t-1} = W_fc h1_{t-1} + b_fc
                        # is folded through the BG column as the rank-1 weight
                        # wbgfc = outer(W_fc, W_bg) applied to last-step h1 —
                        # no fc -> copy -> feedback chain in the recurrence
                        parts.append(
                            (wbgfc, lambda st: h[1][1 - p][:, st, :])
                        )
                else:
                    # h-part first: it only needs last-step h1 (prefetchable);
                    # the x-part (= this step's h0) is the late dependency
                    parts = [(w_h, h_sl), (w_x, x_sl)]

                # g first (it unblocks the DVE chain), then i, f, o
                gps = {}
                for g, tag in [(GG, "psA"), (GI, "psB"), (GF, "psA"), (GO, "psB")]:
                    gps[g] = psum.tile(
                        [H, NS, SB], f32, tag=tag, name=f"ps_{t}_{layer}_{g}"
                    )
                for pi, (w, rhs_fn) in enumerate(parts):
                    for g in (GG, GI, GF, GO):
                        for st in range(NS):
                            nc.tensor.matmul(
                                gps[g][:, st, :], w[:, ts(g, H)], rhs_fn(st),
                                start=pi == 0, stop=pi == len(parts) - 1,
                            )
                return gps

            def layer_act_dve(t, layer, gps):
                p = t % 2
                dec = t >= T_ENC
                h_new = h[layer][p]
                c_own = c[layer]

                ifo_sb = gates.tile([H, 3, NS, SB], f16, tag="ifo")
                g_sb = gates.tile([H, NS, SB], f16, tag="g")
                t1 = gates.tile([H, NS, SB], f16, tag="t1")
                u = gates.tile([H, NS, SB], f16, tag="u")
                th = gates.tile([H, NS, SB], f16, tag="th")
                bias = {}
                if layer == 1:
                    bias = {
                        g: {"bias": b1[:, g : g + 1]} for g in (GI, GF, GG, GO)
                    }

                for lo, hi in [(0, NS)]:
                    sl = slice(lo, hi)
                    nc.scalar.activation(
                        g_sb[:, sl], gps[GG][:, sl], TANH, **bias.get(GG, {})
                    )
                    nc.scalar.activation(
                        ifo_sb[:, 0, sl], gps[GI][:, sl], SIG, **bias.get(GI, {})
                    )
                    nc.scalar.activation(
                        ifo_sb[:, 1, sl], gps[GF][:, sl], SIG, **bias.get(GF, {})
                    )
                    nc.vector.tensor_mul(t1[:, sl], ifo_sb[:, 0, sl], g_sb[:, sl])
                    nc.vector.tensor_mul(u[:, sl], ifo_sb[:, 1, sl], c_own[:, sl])
                    nc.vector.tensor_add(c_own[:, sl], u[:, sl], t1[:, sl])
                    nc.scalar.activation(th[:, sl], c_own[:, sl], TANH)
                    nc.scalar.activation(
                        ifo_sb[:, 2, sl], gps[GO][:, sl], SIG, **bias.get(GO, {})
                    )
                    nc.vector.tensor_mul(
                        h_new[:, sl], ifo_sb[:, 2, sl], th[:, sl]
                    )

            def fc_block(t, xt, tr):
                td = t - T_ENC
                fc = psum.tile([1, NS, SB], f32, tag="psA", name=f"fc_{t}")
                for st in range(NS):
                    nc.tensor.matmul(
                        fc[:, st, :], wfct[:], h[1][t % 2][:, st, :],
                        start=True, stop=True,
                    )
                # output staging (entirely off the recurrence): +b_fc, DMA out
                pred = gates.tile([1, NS, SB], f32, tag="pred")
                nc.vector.tensor_scalar_add(pred[:], fc[:], bfc)
                nc.sync.dma_start(out_d.ap()[td : td + 1, :], pred[:])

            # x tiles: 8-step tiles for encode, one 12-step tile for the
            # whole decode window (so the BG feedback write target exists)
            xt = None
            x_tile_starts = list(range(0, T_ENC, XT_STEPS)) + [T_ENC]
            for t in range(T):
                if t in x_tile_starts:
                    t0 = t
                    nt = T_DEC if t == T_ENC else XT_STEPS
                    xt = xpool.tile([DIN + 1, T_DEC, BSH], bf16)
                    nc.sync.dma_start(
                        xt[:, :nt, :], x_d.ap()[:, t : t + nt, :]
                    )
                tr = t - t0
                for layer in range(2):
                    gps = layer_mm(t, layer, xt, tr)
                    layer_act_dve(t, layer, gps)
                if t >= T_ENC:
                    fc_block(t, xt, tr)

    nc.compile()
    return nc


def _get_nc(bfc: float):
    if _CACHE.get("bfc") != bfc:
        _CACHE["nc"] = _build(bfc)
        _CACHE["bfc"] = bfc
    return _CACHE["nc"]


def kernel(
    inputs,
    W_ih_0, W_hh_0, b_ih_0, b_hh_0,
    W_ih_1, W_hh_1, b_ih_1, b_hh_1,
    W_fc, b_fc,
):
    inputs = np.asarray(inputs, np.float32)
    bfc = float(np.asarray(b_fc).reshape(-1)[0])
    nc = _get_nc(bfc)

    b0 = (b_ih_0 + b_hh_0).astype(np.float32)
    bfc32 = np.float32(bfc)
    w9t0 = np.concatenate(
        [W_ih_0.T.astype(np.float32), b0[None, :]], axis=0
    ).astype(BF16)  # [9, 512]; row 8 is the bias
    # decode variant: bias row also carries b_fc * W_bg (the feedback copy
    # delivers the raw fc output, without b_fc)
    w9dec = np.concatenate(
        [W_ih_0.T.astype(np.float32),
         (b0 + bfc32 * W_ih_0[:, 0].astype(np.float32))[None, :]], axis=0
    ).astype(BF16)
    whht0 = W_hh_0.T.astype(BF16)
    wiht1 = W_ih_1.T.astype(BF16)
    whht1 = W_hh_1.T.astype(BF16)
    wbg0t = W_ih_0.T[0:1, :].astype(BF16)  # BG column of W_ih_0
    # rank-1 fold of the fc head through the BG column: gate j gets
    # W_ih_0[j,0] * (W_fc . h1); lhsT[k, j] = W_fc[0,k] * W_ih_0[j,0]
    wbgfc = np.outer(
        W_fc.astype(np.float32)[0], W_ih_0[:, 0].astype(np.float32)
    ).astype(BF16)  # [128, 512]
    wfct = W_fc.T.astype(BF16)  # [128, 1]
    b1 = (b_ih_1 + b_hh_1).reshape(4, H).T.astype(np.float32)  # [128, 4]

    in_maps = []
    for i in range(N_CORES):
        sh = inputs[i * BSH : (i + 1) * BSH]  # [1024, 60, 8]
        x = np.ascontiguousarray(sh.transpose(2, 1, 0))  # [8, 60, 1024]
        x9 = np.concatenate(
            [x, np.ones((1, T, BSH), np.float32)], axis=0
        )  # [9, 60, 1024]
        x9[0, T_ENC:, :] = 0.0  # BG channel rides the feedback matmul in decode
        bg0 = sh[:, T_ENC, 0].reshape(1, BSH)
        in_maps.append(
            {
                "x": x9.astype(BF16),
                "w9t0": w9t0,
                "w9dec": w9dec,
                "whht0": whht0,
                "wiht1": wiht1,
                "whht1": whht1,
                "wbg0t": wbg0t,
                "wbgfc": wbgfc,
                "wfct": wfct,
                "b1": b1,
                "bg0": bg0.astype(BF16),
            }
        )

    res = bass_utils.run_bass_kernel_spmd(
        nc, in_maps, core_ids=list(range(N_CORES))
    )
    outs = []
    for i in range(N_CORES):
        o = res.results[i]["out"]  # [12, 1024] fp32, b_fc already added
        outs.append(o.T[:, :, None])  # [1024, 12, 1]
    return np.concatenate(outs, axis=0).astype(np.float32)


if __name__ == "__main__":
    _get_nc(0.0)
    print("build + compile OK")
